# revision 60
# baseline (speedup 1.0000x reference)
"""HGAT (2-layer hyperbolic graph attention) Trainium2 kernel, 8-core SPMD.

Sharding: nodes (rows of x/adj) split 8 ways. Per layer the [N,128] tangent
features are all-gathered; softmax rows are local.

Attention decomposition (exact): with s_ij = el_i + er_j,
  exp(leaky_relu(s)) = 1{s>0} e^{el_i} e^{er_j} + 1{s<=0} e^{.2 el_i} e^{.2 er_j}
so  att-weighted agg = [u+ . (A+ @ v+) + u- . (A- @ v-)] row-normalized, where
  A+ = adj * 1{s>0} (bf16 0/1 mask, TS add + TT is_gt vs a resident bf16
  threshold matrix thrT = 256*(1-adj^T)), A-@v- recovered as
  colsum(v-) - thr@v-/256 - A+@v-, and v+/- = [e^{er} * xt | e^{er}].

Pipeline/overlap structure (what makes it fast):
- thr loads on the Pool DGE queue (the greedy ready-time scheduler would
  otherwise put them ahead of the x load / ACT work on any queue they share).
- The per-layer AllGather is split in two halves (xt sent in fp8 + er f32,
  132 B/node) and each half's mask-build + 16 masked matmuls are issued
  immediately after that half's gather, so the second transfer hides under
  the first half's compute.
- All partition broadcasts (el, bias, al/ar) run as PE ones-column matmuls —
  gpsimd.partition_broadcast would queue on Pool behind the collectives.
- colsum(v-) via DVE strided reduce + gpsimd partition_all_reduce (no PSUM
  bank, which lets all 8 accumulator banks live across the chunk loop).
- sqrt is DVE-only (bit-trick + 2 Newton steps) and atanh is Ln-free
  (exponent/mantissa split + cubic + one Exp-Newton step), so the ACT engine
  never leaves the Exp table set (each set switch costs a 1.28us reload).
- Pointwise hyperbolic chains operate on contiguous [128, NT*128] tiles with
  batched Square+reduce norms; |p| entering each mobius_matvec is propagated
  analytically (clipped tanh norm) instead of recomputed.
"""
import sys
import numpy as np

sys.path.insert(0, "/opt/trn_rl_repo")
sys.path.insert(0, "/opt/trn_rl_repo/concourse")

import ml_dtypes
from contextlib import ExitStack

import concourse.bass as bass
import concourse.tile as tile
from concourse import bacc, bass_isa, mybir
from concourse import bass_utils

F32 = mybir.dt.float32
BF16 = mybir.dt.bfloat16
FP8 = mybir.dt.float8e4
AF = mybir.ActivationFunctionType
OP = mybir.AluOpType

N = 8192
D = 128
NC = 8
R = N // NC          # 1024 local rows
NT = R // 128        # 8 row tiles
NCHUNK = N // 128    # 64 j-chunks
MIN = 1e-15
ATANH_CLIP = 1.0 - 1e-5
MAXN = 1.0 - 4e-3
BIG = 256.0
GW = 132             # gather row width (fp8 bytes): 128 xt + 4 (er as f32)


I32 = mybir.dt.int32
LN2 = 0.6931472
# cubic minimax-ish fit of ln(1+t) on [0,1): max abs err ~9e-4 (one Exp-Newton
# step below squares it away)
LC1, LC2, LC3 = 0.98669235, -0.40741606, 0.11478294


def _sqrt(nc, pool, out, in2):
    """out = max(sqrt(in2), MIN), DVE-only (bit-trick init + 2 Newton steps).
    No ACT table involved — keeps the whole kernel on the single Exp act set
    (each Ln-table visit costs a 1.28us LoadActFuncSet round trip)."""
    p, k = in2.shape
    y = pool.tile([p, k], F32, tag="sq_y")
    nc.vector.tensor_scalar(
        y[:].bitcast(I32), in2[:].bitcast(I32), 1, None, OP.arith_shift_right
    )
    nc.vector.tensor_scalar(
        y[:].bitcast(I32), y[:].bitcast(I32), 0x1FBD1DF5, None, OP.add
    )
    ry = pool.tile([p, k], F32, tag="sq_ry")
    zy = pool.tile([p, k], F32, tag="sq_zy")
    for it in range(2):
        nc.vector.reciprocal(ry[:], y[:])
        nc.vector.tensor_tensor(zy[:], in2[:], ry[:], OP.mult)
        nc.vector.tensor_tensor(zy[:], y[:], zy[:], OP.add)
        nc.vector.tensor_scalar(y[:], zy[:], 0.5, None, OP.mult)
    nc.vector.tensor_scalar(out[:], y[:], MIN, None, OP.max)


def _tanh(nc, pool, out, x):
    """out = tanh(x) = 1 - 2/(exp(2x)+1), x >= 0 — Exp-table only."""
    p, k = x.shape
    e = pool.tile([p, k], F32, tag="th_e")
    nc.scalar.activation(e[:], x[:], AF.Exp, scale=2.0)
    d = pool.tile([p, k], F32, tag="th_d")
    nc.vector.tensor_scalar(d[:], e[:], 1.0, None, OP.add)
    r = pool.tile([p, k], F32, tag="th_r")
    nc.vector.reciprocal(r[:], d[:])
    nc.vector.tensor_scalar(out[:], r[:], -2.0, 1.0, OP.mult, OP.add)


def _atanh(nc, pool, out, c):
    """out = atanh(c) = 0.5*ln((1+c)/(1-c)); c in [0, 1-1e-5]. [p, k].
    Ln-free: exponent/mantissa bit split + cubic + one Exp-Newton step
    (x <- x - 1 + r*e^-x). The only ACT op is Exp — no table switch."""
    p, k = c.shape
    ap1 = pool.tile([p, k], F32, tag="at_ap")
    nc.vector.tensor_scalar(ap1[:], c[:], 1.0, None, OP.add)
    am1 = pool.tile([p, k], F32, tag="at_am")
    nc.vector.tensor_scalar(am1[:], c[:], -1.0, 1.0, OP.mult, OP.add)
    inv = pool.tile([p, k], F32, tag="at_inv")
    nc.vector.reciprocal(inv[:], am1[:])
    ratio = pool.tile([p, k], F32, tag="at_ratio")
    nc.vector.tensor_tensor(ratio[:], ap1[:], inv[:], OP.mult)
    # ln(ratio), ratio in [1, ~2e5]
    e_i = pool.tile([p, k], I32, tag="at_ei")
    nc.vector.tensor_scalar(
        e_i[:], ratio[:].bitcast(I32), 23, None, OP.arith_shift_right
    )
    e_f = pool.tile([p, k], F32, tag="at_ef")
    nc.vector.tensor_copy(e_f[:], e_i[:])  # int -> float value cast
    nc.vector.tensor_scalar(e_f[:], e_f[:], -127.0, None, OP.add)
    m_f = pool.tile([p, k], F32, tag="at_mf")
    nc.vector.tensor_scalar(
        m_f[:].bitcast(I32), ratio[:].bitcast(I32), 0x007FFFFF, 0x3F800000,
        OP.bitwise_and, OP.bitwise_or,
    )
    t = pool.tile([p, k], F32, tag="at_t")
    nc.vector.tensor_scalar(t[:], m_f[:], -1.0, None, OP.add)
    q = pool.tile([p, k], F32, tag="at_q")
    nc.vector.tensor_scalar(q[:], t[:], LC3, LC2, OP.mult, OP.add)
    nc.vector.tensor_tensor(q[:], q[:], t[:], OP.mult)
    nc.vector.tensor_scalar(q[:], q[:], LC1, None, OP.add)
    nc.vector.tensor_tensor(q[:], q[:], t[:], OP.mult)  # q = cubic(t)
    x0 = pool.tile([p, k], F32, tag="at_x0")
    nc.vector.scalar_tensor_tensor(x0[:], e_f[:], LN2, q[:], OP.mult, OP.add)
    en = pool.tile([p, k], F32, tag="at_en")
    nc.scalar.activation(en[:], x0[:], AF.Exp, scale=-1.0)
    re_n = pool.tile([p, k], F32, tag="at_rn")
    nc.vector.tensor_tensor(re_n[:], ratio[:], en[:], OP.mult)
    nc.vector.tensor_tensor(re_n[:], x0[:], re_n[:], OP.add)
    nc.vector.tensor_scalar(out[:], re_n[:], 0.5, -0.5, OP.mult, OP.add)


def _dot_cols(nc, pool, out_col, a, b_t):
    """out_col [p,1] = sum over free dim of a*b_t, on DVE.
    (tensor_tensor_reduce crashes the runtime; use STT with accum_out.)"""
    p = a.shape[0]
    k = int(np.prod(a.shape[1:]))
    scratch = pool.tile([p, k], F32, tag="dot_scr")
    nc.vector.scalar_tensor_tensor(
        scratch[:], a[:], 1.0, b_t[:], OP.mult, OP.mult, accum_out=out_col[:]
    )


def _dot_self(nc, pool, out_col, a):
    """out_col [p,1] = sum(a*a) along free dim — on ACT (Square + accumulate),
    freeing DVE."""
    p = a.shape[0]
    k = int(np.prod(a.shape[1:]))
    scratch = pool.tile([p, k], F32, tag="dot_scr2")
    nc.scalar.activation(scratch[:], a[:], AF.Square, accum_out=out_col[:])


def build_program():
    import os
    stop = int(os.environ.get("HGAT_STOP", "9"))
    nc = bacc.Bacc(
        "TRN2", target_bir_lowering=False, debug=False, num_devices=NC
    )
    # raise the tile allocator SBUF cap (stale 192KB constant; trn2 has 208KB usable)
    try:
        from concourse import tile_utils
        tile_utils.max_sbuf_usage = 206 * 1024
    except Exception:
        pass
    try:
        tile.max_sbuf_usage = 206 * 1024
    except Exception:
        pass

    x_in = nc.dram_tensor("x_shard", [R, D], F32, kind="ExternalInput").ap()
    thr_in = nc.dram_tensor("thr_shard", [N, R], BF16, kind="ExternalInput").ap()
    wts = {}
    for li in (1, 2):
        wts[f"WT{li}"] = nc.dram_tensor(f"WT{li}", [D, D], F32, kind="ExternalInput").ap()
        for v in ("b", "al", "ar"):
            wts[f"{v}{li}"] = nc.dram_tensor(f"{v}{li}", [1, D], F32, kind="ExternalInput").ap()
    ident_f32 = nc.dram_tensor("ident_f32", [128, 128], F32, kind="ExternalInput").ap()
    y_out = nc.dram_tensor("y_shard", [R, D], F32, kind="ExternalOutput").ap()

    with tile.TileContext(nc) as tc, ExitStack() as ctx:
        # ---------------- pools ----------------
        big = ctx.enter_context(tc.tile_pool(name="big", bufs=1))      # residents
        sp = ctx.enter_context(tc.tile_pool(name="scal", bufs=1))      # small scalar tiles
        work = ctx.enter_context(tc.tile_pool(name="work", bufs=1))    # [128,128] f32 work
        dram = ctx.enter_context(tc.tile_pool(name="dram", bufs=1, space="DRAM"))

        # ---------------- constants ----------------
        idf = big.tile([128, 128], F32, tag="idf")
        nc.sync.dma_start(idf[:], ident_f32[:])
        ones_f32 = sp.tile([1, 128], F32, tag="ones_f32")
        nc.vector.memset(ones_f32[:], 1.0)

        _bc_n = [0]

        def _pe_bcast(dst_sb, src_row):
            """Broadcast [1, n] f32 along partitions via a ones-column matmul
            (PE + DVE evict). Keeps the Pool queue free — gpsimd
            partition_broadcast gets stuck behind the thr load / collectives
            under the greedy ready-time scheduler."""
            n = dst_sb.shape[-1]
            _bc_n[0] += 1
            with tc.tile_pool(name=f"ps_bc{_bc_n[0]}", bufs=1, space="PSUM") as pb:
                ps = pb.tile([128, n], F32, tag="bc")
                nc.tensor.matmul(
                    ps[:], ones_f32[:], src_row[:], start=True, stop=True
                )
                nc.vector.tensor_copy(dst_sb[:], ps[:])

        # hoist the x load ahead of the big thr transfer so the encode
        # phase starts immediately (sync queue is FIFO); all 8 row tiles
        # live in one contiguous [128, NT, D] tile so norms batch into
        # single wide ops
        P0 = work.tile([128, NT, D], F32, tag="P0")
        nc.sync.dma_start(P0[:], x_in[:].rearrange("(t p) d -> p t d", p=128))


        # ------- thr = 256*(1-adj^T) bf16, host-prepped, loaded resident -------
        # thr_all[p, c, i] = 256*(1 - adj[i_global, c*128+p])
        # spread across 4 engine DGE queues so the quarters transfer in parallel
        # all quarters on the Pool queue: Pool has nothing else to do before
        # the first collective (~57us), while the sync queue must serve the
        # x load + small DMAs immediately (the scheduler orders by readiness,
        # and thr is ready at t=0 — on sync it would starve the encode)
        thr_all = big.tile([128, NCHUNK, R], BF16, tag="thr")
        for q in range(4):
            nc.gpsimd.dma_start(
                thr_all[:, q * 16:(q + 1) * 16, :],
                thr_in[q * 16 * 128:(q + 1) * 16 * 128, :]
                .rearrange("(c p) i -> p c i", p=128),
            )

        stage = ctx.enter_context(tc.tile_pool(name="stage", bufs=1))
        mpool = ctx.enter_context(tc.tile_pool(name="masks", bufs=2))

        def early_out(tiles):
            for tt_, tl in enumerate(tiles):
                cp = work.tile([128, D], F32, tag="eo", bufs=2, name=f"eo{tt_}")
                nc.vector.tensor_copy(cp[:], tl[:] if tl.shape[-1] == D else tl[:, 0:D])
                nc.sync.dma_start(y_out[tt_ * 128:(tt_ + 1) * 128, :], cp[:])

        if stop == 1:
            early_out([thr_all[:, c, 0:D] for c in range(NT)])

        # ---------------- encode: p = proj(expmap0(x)) ----------------
        run_layers = stop >= 2
        p_tiles = []
        def _norms2(big_ap, out_col, name):
            """out_col [128, NT] = per-tile squared norms of [128, NT, D]
            big_ap — ONE wide ACT Square (into the idle mask-pool buffer,
            same byte size) + ONE DVE free-dim reduce, instead of 8 per-tile
            Square+accumulate ops."""
            sq_flat = mpool.tile([128, R], BF16, tag="sc", name=f"sq_{name}")
            sq = sq_flat[:].rearrange("p (t d) -> p t d", t=NT)
            nc.scalar.activation(sq, big_ap, AF.Square)
            nc.vector.tensor_reduce(out_col[:], sq, mybir.AxisListType.X, OP.add)

        if stop >= 2:
            n2 = sp.tile([128, NT], F32, tag="enc_n2")
            _norms2(P0[:], n2, "enc")
            nrm = sp.tile([128, NT], F32, tag="enc_n")
            _sqrt(nc, sp, nrm, n2)
            th = sp.tile([128, NT], F32, tag="enc_th")
            _tanh(nc, sp, th, nrm)
            thc = sp.tile([128, NT], F32, tag="enc_thc")
            nc.vector.tensor_scalar(thc[:], th[:], MAXN, None, OP.min)
            invn = sp.tile([128, NT], F32, tag="enc_invn")
            nc.vector.reciprocal(invn[:], nrm[:])
            sc = sp.tile([128, NT], F32, tag="enc_sc")
            nc.vector.tensor_tensor(sc[:], thc[:], invn[:], OP.mult)
            p_norm = thc  # |p| after the scale, used by layer 1's mobius
            for t in range(NT):
                nc.vector.tensor_scalar(
                    P0[:, t, :], P0[:, t, :], sc[:, t:t + 1], None, OP.mult
                )
            p_tiles = [P0[:, t, :] for t in range(NT)]

        if stop == 2:
            early_out(p_tiles)
            run_layers = False

        # ---------------- two HGAT layers ----------------
        for li in (1, 2) if run_layers else ():
            WT_sb = big.tile([128, 128], F32, tag="WT")
            nc.sync.dma_start(WT_sb[:], wts[f"WT{li}"][:])
            b_sb = sp.tile([1, D], F32, tag="b_sb")
            nc.sync.dma_start(b_sb[:], wts[f"b{li}"][:])
            al_sb = sp.tile([1, D], F32, tag="al_sb")
            nc.sync.dma_start(al_sb[:], wts[f"al{li}"][:])
            ar_sb = sp.tile([1, D], F32, tag="ar_sb")
            nc.sync.dma_start(ar_sb[:], wts[f"ar{li}"][:])

            # ---- mobius_matvec: h = proj(tanh(mxn/xn*atanh(clip(xn))) mx/mxn)
            # |p| is known analytically from the producing stage (clipped
            # tanh norm, p_norm) — no dot/sqrt needed for xn.
            xn = p_norm
            MX = work.tile([128, NT, D], F32, tag="MX")
            mx_tiles = []
            with tc.tile_pool(name=f"ps_w{li}", bufs=2, space="PSUM") as psw:
                for t in range(NT):
                    ptp = psw.tile([128, 128], F32, tag="ptp")
                    nc.tensor.transpose(ptp[:], p_tiles[t][:], idf[:])
                    pT = work.tile([128, 128], F32, tag="pT", bufs=2)
                    nc.vector.tensor_copy(pT[:], ptp[:])
                    mxp = psw.tile([128, 128], F32, tag="mxp")
                    nc.tensor.matmul(mxp[:], pT[:], WT_sb[:], start=True, stop=True)
                    nc.vector.tensor_copy(MX[:, t, :], mxp[:])
                    mx_tiles.append(MX[:, t, :])
            mxn2 = sp.tile([128, NT], F32, tag="mxn2")
            _norms2(MX[:], mxn2, "mxn")
            mxn = sp.tile([128, NT], F32, tag="mxn")
            _sqrt(nc, sp, mxn, mxn2)
            cx = sp.tile([128, NT], F32, tag="cx")
            nc.vector.tensor_scalar(cx[:], xn[:], ATANH_CLIP, None, OP.min)
            at = sp.tile([128, NT], F32, tag="at")
            _atanh(nc, sp, at, cx)
            ixn = sp.tile([128, NT], F32, tag="ixn")
            nc.vector.reciprocal(ixn[:], xn[:])
            q = sp.tile([128, NT], F32, tag="q")
            nc.vector.tensor_tensor(q[:], at[:], ixn[:], OP.mult)
            arg = sp.tile([128, NT], F32, tag="arg")
            nc.vector.tensor_tensor(arg[:], q[:], mxn[:], OP.mult)
            thm = sp.tile([128, NT], F32, tag="thm")
            _tanh(nc, sp, thm, arg)
            imxn = sp.tile([128, NT], F32, tag="imxn")
            nc.vector.reciprocal(imxn[:], mxn[:])
            # proj scale on result (|res| = thm): min(thm, MAXN)
            thmc = sp.tile([128, NT], F32, tag="thmc")
            nc.vector.tensor_scalar(thmc[:], thm[:], MAXN, None, OP.min)
            lam = sp.tile([128, NT], F32, tag="lam")
            nc.vector.tensor_tensor(lam[:], thmc[:], imxn[:], OP.mult)
            for t in range(NT):
                nc.scalar.activation(
                    mx_tiles[t][:], mx_tiles[t][:], AF.Copy, scale=lam[:, t:t + 1]
                )
            h_tiles = mx_tiles

            # ---- bh = proj(expmap0(b)), broadcast ----
            bn2 = sp.tile([1, 1], F32, tag="bn2")
            _dot_self(nc, sp, bn2, b_sb)
            bn = sp.tile([1, 1], F32, tag="bn")
            _sqrt(nc, sp, bn, bn2)
            bth = sp.tile([1, 1], F32, tag="bth")
            _tanh(nc, sp, bth, bn)
            bthc = sp.tile([1, 1], F32, tag="bthc")
            nc.vector.tensor_scalar(bthc[:], bth[:], MAXN, None, OP.min)
            ibn = sp.tile([1, 1], F32, tag="ibn")
            nc.vector.reciprocal(ibn[:], bn[:])
            bsc = sp.tile([1, 1], F32, tag="bsc")
            nc.vector.tensor_tensor(bsc[:], bthc[:], ibn[:], OP.mult)
            bh = sp.tile([1, D], F32, tag="bh")
            nc.vector.tensor_scalar(bh[:], b_sb[:], bsc[:], None, OP.mult)
            bh_b = big.tile([128, D], F32, tag="bh_b")
            _pe_bcast(bh_b, bh)
            bh2 = sp.tile([1, 1], F32, tag="bh2")
            _dot_self(nc, sp, bh2, bh)
            bh2_b = sp.tile([128, 1], F32, tag="bh2_b")
            _pe_bcast(bh2_b, bh2)

            # ---- mobius_add(h, bh) + proj ----
            # coefficient math batched over all NT tiles as [128, NT] columns;
            # x2 = |h|^2 is analytic (= thmc^2, the clipped tanh norm)
            H2 = work.tile([128, NT, D], F32, tag="H2")
            hn = sp.tile([128, NT], F32, tag="hn")
            hn2s = sp.tile([128, NT], F32, tag="hn2s")
            x2 = sp.tile([128, NT], F32, tag="x2")
            nc.vector.tensor_tensor(x2[:], thmc[:], thmc[:], OP.mult)
            xy = sp.tile([128, NT], F32, tag="xy")
            for t in range(NT):
                _dot_cols(nc, sp, xy[:, t:t + 1], h_tiles[t], bh_b)
            # cf = 1 + 2xy + y2 ; den = 1 + 2xy + x2*y2
            bh2p1 = sp.tile([128, 1], F32, tag="bh2p1")
            nc.vector.tensor_scalar(bh2p1[:], bh2_b[:], 1.0, None, OP.add)
            cf = sp.tile([128, NT], F32, tag="cf")
            nc.vector.tensor_scalar(cf[:], xy[:], 2.0, None, OP.mult)
            nc.vector.tensor_scalar(cf[:], cf[:], bh2p1[:], None, OP.add)
            x2y2 = sp.tile([128, NT], F32, tag="x2y2")
            nc.vector.tensor_scalar(x2y2[:], x2[:], bh2_b[:], None, OP.mult)
            den = sp.tile([128, NT], F32, tag="den")
            nc.vector.scalar_tensor_tensor(
                den[:], xy[:], 2.0, x2y2[:], OP.mult, OP.add
            )
            nc.vector.tensor_scalar(den[:], den[:], 1.0, MIN, OP.add, OP.max)
            iden = sp.tile([128, NT], F32, tag="iden")
            nc.vector.reciprocal(iden[:], den[:])
            cb = sp.tile([128, NT], F32, tag="cb")
            nc.vector.tensor_scalar(cb[:], x2[:], -1.0, 1.0, OP.mult, OP.add)
            h2_tiles = []
            for t in range(NT):
                na = work.tile([128, D], F32, tag="na", bufs=2)
                nc.vector.tensor_scalar(
                    na[:], h_tiles[t][:], cf[:, t:t + 1], None, OP.mult
                )
                nb = work.tile([128, D], F32, tag="nb", bufs=2)
                nc.vector.tensor_scalar(nb[:], bh_b[:], cb[:, t:t + 1], None, OP.mult)
                nc.vector.tensor_tensor(na[:], na[:], nb[:], OP.add)
                nc.vector.tensor_scalar(
                    H2[:, t, :], na[:], iden[:, t:t + 1], None, OP.mult
                )
                h2_tiles.append(H2[:, t, :])
            _norms2(H2[:], hn2s, "hn")
            _sqrt(nc, sp, hn, hn2s)
            ihn = sp.tile([128, NT], F32, tag="ihn")
            nc.vector.reciprocal(ihn[:], hn[:])
            psc = sp.tile([128, NT], F32, tag="psc")
            nc.vector.tensor_scalar(psc[:], ihn[:], MAXN, 1.0, OP.mult, OP.min)
            # h3 = proj(h2); xt = atanh(clip(|h3|))/|h3| * h3
            # |h3| = min(hn, MAXN); xt = h2 * psc * atanh(clip(min(hn,MAXN)))/min(hn,MAXN)
            hnc = sp.tile([128, NT], F32, tag="hnc")
            nc.vector.tensor_scalar(hnc[:], hn[:], MAXN, ATANH_CLIP, OP.min, OP.min)
            ath = sp.tile([128, NT], F32, tag="ath")
            _atanh(nc, sp, ath, hnc)
            # xt = h2 * (psc * ath / min(hn, MAXN)) ; note psc/min(hn,MAXN) = min(1/hn, ...)
            hm = sp.tile([128, NT], F32, tag="hm")
            nc.vector.tensor_scalar(hm[:], hn[:], MAXN, None, OP.min)
            ihm = sp.tile([128, NT], F32, tag="ihm")
            nc.vector.reciprocal(ihm[:], hm[:])
            xsc = sp.tile([128, NT], F32, tag="xsc")
            nc.vector.tensor_tensor(xsc[:], psc[:], ihm[:], OP.mult)
            nc.vector.tensor_tensor(xsc[:], xsc[:], ath[:], OP.mult)
            for t in range(NT):
                nc.scalar.activation(
                    h2_tiles[t][:], h2_tiles[t][:], AF.Copy, scale=xsc[:, t:t + 1]
                )
            xt_tiles = h2_tiles

            if stop == 3 and li == 1:
                early_out(xt_tiles)
                break

            # ---- el, er_local ----
            al_b = big.tile([128, D], F32, tag="al_b")
            _pe_bcast(al_b, al_sb)
            ar_b = big.tile([128, D], F32, tag="ar_b")
            _pe_bcast(ar_b, ar_sb)
            el = sp.tile([128, NT], F32, tag="el")
            erl = sp.tile([128, NT], F32, tag="erl")
            for t in range(NT):
                _dot_cols(nc, sp, el[:, t:t + 1], xt_tiles[t], al_b)
                _dot_cols(nc, sp, erl[:, t:t + 1], xt_tiles[t], ar_b)

            # ---- local-only prep (no gather dependency): up/um, el broadcast ----
            up = sp.tile([128, NT], F32, tag="up")
            nc.scalar.activation(up[:], el[:], AF.Exp)
            um = sp.tile([128, NT], F32, tag="um")
            nc.scalar.activation(um[:], el[:], AF.Exp, scale=0.2)
            el_dram = dram.tile([1, R], F32, tag=f"eld{li}")
            nc.sync.dma_start(
                el_dram[:].rearrange("one (t p) -> (one p) t", p=128), el[:]
            )
            el_row = stage.tile([1, R], F32, tag="el_row")
            nc.sync.dma_start(el_row[:], el_dram[:])
            # broadcast el along partitions on the PE (ones-column matmul).
            # gpsimd.partition_broadcast would queue on Pool BEHIND the two
            # collectives (greedy ready-time ordering) and stall every mask
            # build by ~30us.
            el_b = big.tile([128, R], BF16, tag="el_b")
            with tc.tile_pool(name=f"ps_el{li}", bufs=1, space="PSUM") as pse:
                for hlf in range(2):
                    el_ps = pse.tile([128, R // 2], F32, tag="el_ps", bufs=2)
                    nc.tensor.matmul(
                        el_ps[:], ones_f32[:],
                        el_row[:, hlf * (R // 2):(hlf + 1) * (R // 2)],
                        start=True, stop=True,
                    )
                    nc.vector.tensor_copy(
                        el_b[:, hlf * (R // 2):(hlf + 1) * (R // 2)], el_ps[:]
                    )

            # ---- build + AllGather [xt_bf16 | 1 | pad | er_f32], in 2 halves
            # so the second half's transfer overlaps the first half's compute.
            # Half h covers local tiles h*4..h*4+3; its gather output rows
            # (k*512 + q*128 + p) hold node j = k*1024 + h*512 + q*128 + p,
            # i.e. global j-chunk c = k*8 + h*4 + q  (local block m = k*4+q).
            gaths = {}
            for h in (0, 1):
                send = dram.tile([R // 2, GW], FP8, tag=f"send{li}_{h}",
                                 name=f"send{li}_{h}")
                gath = dram.tile([N // 2, GW], FP8, tag=f"gath{li}_{h}",
                                 addr_space="Shared", name=f"gath{li}_{h}")
                for tq in range(4):
                    t = h * 4 + tq
                    sb_send = stage.tile([128, GW], FP8, tag="sb_send", bufs=2)
                    nc.vector.tensor_copy(sb_send[:, 0:D], xt_tiles[t][:])
                    nc.vector.tensor_copy(
                        sb_send[:].bitcast(F32)[:, 32:33], erl[:, t:t + 1]
                    )
                    nc.sync.dma_start(send[tq * 128:(tq + 1) * 128, :], sb_send[:])
                nc.gpsimd.collective_compute(
                    "AllGather",
                    OP.bypass,
                    replica_groups=[list(range(NC))],
                    ins=[send[:]],
                    outs=[gath[:]],
                )
                gaths[h] = gath

            def c_of(h, m):
                # local gather block m (= k*4+q) -> global j-chunk index
                return (m // 4) * 8 + h * 4 + (m % 4)

            # ---- per half: er/exp/v-build/colsum, then that half's masked
            # matmul chunk loop — issue order matches data-arrival order so
            # half 1's transfer hides behind half 0's compute (engine queues
            # are strict FIFO; issuing h1 builds early would head-of-line
            # block the DVE on the h1 gather).
            NH = NCHUNK // 2  # 32 local blocks per half
            W2C = 2 * (D + 1)  # 258
            vall = big.tile([128, NCHUNK, W2C], BF16, tag="vall")
            # acc[t] = [ A+@v+ (0:129) | A+@v- (129:258) | thr@v- (258:387) ]
            psa = tc.alloc_tile_pool(name=f"ps_acc{li}", bufs=1, space="PSUM")
            acc = [
                psa.tile([128, 3 * (D + 1)], F32, tag=f"acc{t}", name=f"acc{t}_l{li}")
                for t in range(NT)
            ]
            with tc.tile_pool(name=f"xg{li}", bufs=1) as xgp:
                for h in (0, 1):
                    gath_f32 = gaths[h][:].bitcast(F32)  # [N/2, 33]
                    er_a = sp.tile([128, NH], F32, tag=f"er_all{h}",
                                   name=f"er_all{li}_{h}")
                    nc.sync.dma_start(
                        er_a[:],
                        gath_f32[:, 32:33].rearrange("(c p) one -> p (c one)", p=128),
                    )
                    wp = sp.tile([128, NH], F32, tag=f"wp{h}", name=f"wp{li}_{h}")
                    nc.scalar.activation(wp[:], er_a[:], AF.Exp)
                    wm = sp.tile([128, NH], F32, tag=f"wm{h}", name=f"wm{li}_{h}")
                    nc.scalar.activation(wm[:], er_a[:], AF.Exp, scale=0.2)

                    # ---- masked attention matmuls for this half; the
                    # per-chunk v build is inlined so DVE alternates
                    # [v-build, mask] instead of a long v-build burst
                    # delaying the first mask.
                    for m in range(NH):
                        c = c_of(h, m)
                        k = m % 16
                        if k == 0:
                            gblk = m // 16
                            xtg = xgp.tile([128, 16, D], FP8, tag="xtg",
                                           name=f"xtg{li}_{h}_{gblk}")
                            nc.sync.dma_start(
                                xtg[:],
                                gaths[h][gblk * 16 * 128:(gblk + 1) * 16 * 128,
                                         0:D]
                                .rearrange("(c p) w -> p c w", p=128),
                            )
                        nc.vector.tensor_scalar(
                            vall[:, c, 0:D], xtg[:, k, :], wp[:, m:m + 1],
                            None, OP.mult,
                        )
                        nc.vector.tensor_copy(vall[:, c, D:D + 1], wp[:, m:m + 1])
                        nc.vector.tensor_scalar(
                            vall[:, c, D + 1:W2C - 1], xtg[:, k, :], wm[:, m:m + 1],
                            None, OP.mult,
                        )
                        nc.vector.tensor_copy(
                            vall[:, c, W2C - 1:W2C], wm[:, m:m + 1]
                        )
                        s_c = mpool.tile([128, R], BF16, tag="sc")
                        nc.vector.tensor_scalar(
                            s_c[:], el_b[:], er_a[:, m:m + 1], None, OP.add
                        )
                        ap_ = mpool.tile([128, R], BF16, tag="Ap")
                        nc.vector.tensor_tensor(
                            ap_[:], s_c[:], thr_all[:, c, :], OP.is_gt
                        )
                        # One accumulation group per PSUM bank: start only on
                        # the very first matmul into the bank, stop on the last.
                        first = h == 0 and m == 0
                        last = h == 1 and m == NH - 1
                        for t in range(NT):
                            nc.tensor.matmul(
                                acc[t][:, 0:W2C],
                                ap_[:, t * 128:(t + 1) * 128],
                                vall[:, c, :],
                                start=first, stop=False,
                            )
                            nc.tensor.matmul(
                                acc[t][:, W2C:W2C + D + 1],
                                thr_all[:, c, t * 128:(t + 1) * 128],
                                vall[:, c, D + 1:W2C],
                                start=False, stop=last,
                            )
                # CS = colsum(v-) without burning a PSUM bank: DVE free-dim
                # reduce over chunks, then gpsimd partition all-reduce
                # (result broadcast to all partitions). Issued after half 1's
                # v builds; consumed only at eviction.
                if h == 1:
                    cs_part = sp.tile([128, D + 1], F32, tag="cs_part")
                    nc.vector.tensor_reduce(
                        cs_part[:],
                        vall[:, :, D + 1:W2C].rearrange("p c f -> p f c"),
                        mybir.AxisListType.X, OP.add,
                    )
                    csb = big.tile([128, D + 1], F32, tag="csb")
                    nc.gpsimd.partition_all_reduce(
                        csb[:], cs_part[:], 128, bass_isa.ReduceOp.add
                    )
            if stop == 4 and li == 1:
                early_out([vall[:, c, 0:D] for c in range(NT)])
                break

            # ---- evict + normalize -> agg ----
            # A-@v- = CS - thr@v-/256 - A+@v-
            AGG = work.tile([128, NT, D], F32, tag="P0")  # reuse P0's space
            agg_tiles = []
            for t in range(NT):
                e1 = work.tile([128, D + 1], F32, tag="e1")
                nc.vector.tensor_scalar(
                    e1[:], acc[t][:, W2C:W2C + D + 1], -1.0 / BIG, None, OP.mult
                )
                nc.vector.scalar_tensor_tensor(
                    e1[:], acc[t][:, D + 1:W2C], -1.0, e1[:], OP.mult, OP.add
                )
                t2 = work.tile([128, D + 1], F32, tag="t2")
                nc.vector.tensor_tensor(t2[:], e1[:], csb[:], OP.add)
                r1 = work.tile([128, D + 1], F32, tag="r1")
                nc.vector.tensor_scalar(
                    r1[:], acc[t][:, 0:D + 1], up[:, t:t + 1], None, OP.mult
                )
                res = work.tile([128, D + 1], F32, tag="res")
                nc.vector.scalar_tensor_tensor(
                    res[:], t2[:], um[:, t:t + 1], r1[:], OP.mult, OP.add,
                )
                dn = sp.tile([128, 1], F32, tag="dn")
                nc.vector.tensor_scalar(dn[:], res[:, D:D + 1], MIN, None, OP.max)
                idn = sp.tile([128, 1], F32, tag="idn")
                nc.vector.reciprocal(idn[:], dn[:])
                nc.scalar.activation(
                    AGG[:, t, :], res[:, 0:D], AF.Copy, scale=idn[:]
                )
                agg_tiles.append(AGG[:, t, :])
            psa.release()

            if stop == 5 and li == 1:
                early_out(agg_tiles)
                break

            # ---- tail: out = proj(expmap0(relu(logmap0(proj(expmap0(agg)))))) ----
            an2 = sp.tile([128, NT], F32, tag="an2")
            _norms2(AGG[:], an2, "an")
            an = sp.tile([128, NT], F32, tag="an")
            _sqrt(nc, sp, an, an2)
            ath2 = sp.tile([128, NT], F32, tag="ath2")
            _tanh(nc, sp, ath2, an)
            # n2 = min(tanh, MAXN); c3 = min(n2, CLIP); xt2 = agg * atanh(c3)/an
            c3 = sp.tile([128, NT], F32, tag="c3")
            nc.vector.tensor_scalar(c3[:], ath2[:], MAXN, ATANH_CLIP, OP.min, OP.min)
            at3 = sp.tile([128, NT], F32, tag="at3")
            _atanh(nc, sp, at3, c3)
            ian = sp.tile([128, NT], F32, tag="ian")
            nc.vector.reciprocal(ian[:], an[:])
            sc3 = sp.tile([128, NT], F32, tag="sc3")
            nc.vector.tensor_tensor(sc3[:], at3[:], ian[:], OP.mult)
            rn2 = sp.tile([128, NT], F32, tag="rn2")
            for t in range(NT):
                nc.vector.tensor_scalar(
                    agg_tiles[t][:], agg_tiles[t][:], sc3[:, t:t + 1], 0.0,
                    OP.mult, OP.max,
                )
            _norms2(AGG[:], rn2, "rn")
            r_tiles = agg_tiles
            rn = sp.tile([128, NT], F32, tag="rn")
            _sqrt(nc, sp, rn, rn2)
            rth = sp.tile([128, NT], F32, tag="rth")
            _tanh(nc, sp, rth, rn)
            rthc = sp.tile([128, NT], F32, tag="rthc")
            nc.vector.tensor_scalar(rthc[:], rth[:], MAXN, None, OP.min)
            irn = sp.tile([128, NT], F32, tag="irn")
            nc.vector.reciprocal(irn[:], rn[:])
            fsc = sp.tile([128, NT], F32, tag="fsc")
            nc.vector.tensor_tensor(fsc[:], rthc[:], irn[:], OP.mult)
            for t in range(NT):
                nc.scalar.activation(
                    r_tiles[t][:], r_tiles[t][:], AF.Copy, scale=fsc[:, t:t + 1]
                )
            p_tiles = r_tiles
            p_norm = rthc  # |out| analytic, feeds layer 2's mobius
            if stop == 6 and li == 1:
                early_out(p_tiles)
                break

        # ---------------- output ----------------
        if stop >= 7:
            for t in range(NT):
                nc.sync.dma_start(y_out[t * 128:(t + 1) * 128, :], p_tiles[t][:])

    nc.compile()
    return nc


_PROGRAM = None


def prep_in_maps(inputs):
    x = np.ascontiguousarray(inputs["x"], np.float32)
    adj = np.asarray(inputs["adj"], np.float32)
    base = {
        "WT1": np.ascontiguousarray(inputs["W1"].T, np.float32),
        "WT2": np.ascontiguousarray(inputs["W2"].T, np.float32),
        "b1": np.asarray(inputs["b1"], np.float32).reshape(1, D),
        "b2": np.asarray(inputs["b2"], np.float32).reshape(1, D),
        "al1": np.asarray(inputs["al1"], np.float32).reshape(1, D),
        "al2": np.asarray(inputs["al2"], np.float32).reshape(1, D),
        "ar1": np.asarray(inputs["ar1"], np.float32).reshape(1, D),
        "ar2": np.asarray(inputs["ar2"], np.float32).reshape(1, D),
        "ident_f32": np.eye(128, dtype=np.float32),
    }
    in_maps = []
    for r in range(NC):
        m = dict(base)
        m["x_shard"] = x[r * R:(r + 1) * R]
        # sharded layout prep: per-core threshold matrix 256*(1-adj_rows)^T, bf16
        m["thr_shard"] = np.ascontiguousarray(
            (256.0 - 256.0 * adj[r * R:(r + 1) * R].T).astype(ml_dtypes.bfloat16)
        )
        in_maps.append(m)
    return in_maps


def kernel(**inputs):
    global _PROGRAM
    if _PROGRAM is None:
        _PROGRAM = build_program()
    nc = _PROGRAM

    in_maps = prep_in_maps(inputs)

    global _last_in_maps
    _last_in_maps = in_maps
    res = bass_utils.run_bass_kernel_spmd(nc, in_maps, core_ids=list(range(NC)))
    return np.concatenate([res.results[r]["y_shard"] for r in range(NC)], axis=0)


_last_in_maps = None


if __name__ == "__main__":
    import reference
    inputs = {k: np.asarray(v) for k, v in reference.setup_inputs().items()}
    out = kernel(**inputs)
    print("out", out.shape, out.dtype)

